# revision 15
# baseline (speedup 1.0000x reference)
"""Trainium2 Bass kernel for top-2-of-8 MoE (T=4096, H=1024, I=1024).

Strategy (tensor-parallel over intermediate dim, 8 cores):
  - Each core gets the full tokens (replicated) + a 128-wide shard of every
    expert's up/down projection (I is sharded 8 ways).
  - Routing (softmax + top-2 + renormalize) is computed on every core from the
    full router logits; it reduces to per-(token, expert) combine weights
    w8[t,e] (nonzero only for the 2 selected experts).
  - Dense-masked grouped GEMM: for each 128-token tile, compute all 8 experts'
    up-proj (tokens stationary), gated-SiLU, scale by w8, transpose, down-proj
    accumulated over experts into PSUM -> per-core partial output [T, H].
  - ReduceScatter(add) over the 8 cores sums the I-shard partials; core r ends
    with rows [r*T/8, (r+1)*T/8) which the host concatenates.

Compute dtype bf16 (f32 PSUM accumulation), f32 output.
"""

import os
import sys

for _p in ("/opt/trn_rl_repo",):
    if _p not in sys.path:
        sys.path.append(_p)

import numpy as np
import ml_dtypes

import concourse.bass as bass
import concourse.bacc as bacc
import concourse.mybir as mybir
import concourse.tile as tile
from concourse.bass_utils import run_bass_kernel_spmd
from concourse.masks import make_identity

BF16 = mybir.dt.bfloat16
F32 = mybir.dt.float32
AX = mybir.AxisListType
OP = mybir.AluOpType
AF = mybir.ActivationFunctionType

N_CORES = 8
H = 1024
I_FULL = 1024
E = 8
K_TOP = 2
IS = I_FULL // N_CORES  # 128, per-core shard of intermediate dim
KT = H // 128  # 8 contraction k-tiles
P = 128


def _rearrange(x, pattern, **kw):
    import einops

    return np.ascontiguousarray(einops.rearrange(x, pattern, **kw))


def build_graph(T):
    """Build the SPMD graph for a T-token problem. Same graph on all cores."""
    NT = T // P  # token tiles
    TS = T // N_CORES  # output rows per core

    nc = bacc.Bacc("TRN2", target_bir_lowering=False, debug=False,
                   num_devices=N_CORES)

    xt_ext = nc.dram_tensor("xt", [P, KT * T], BF16, kind="ExternalInput")
    wup_ext = nc.dram_tensor("wup", [P, KT * E * 256], BF16, kind="ExternalInput")
    wdn_ext = nc.dram_tensor("wdn", [P, E * H], BF16, kind="ExternalInput")
    lg_ext = nc.dram_tensor("lg", [P, NT * E], F32, kind="ExternalInput")
    out_ext = nc.dram_tensor("out", [TS, H], F32, kind="ExternalOutput")

    # internal DRAM for the collective
    rs_in = nc.dram_tensor("rs_in", [T, H], BF16)
    rs_out = nc.dram_tensor("rs_out", [TS, H], BF16)

    with tile.TileContext(nc) as tc:
        with (
            tc.tile_pool(name="big", bufs=1) as big,
            tc.tile_pool(name="work", bufs=3) as work,
            tc.tile_pool(name="outp", bufs=2) as outp,
            tc.tile_pool(name="pup", bufs=1, space="PSUM") as pup,  # 4 tags x 1
            tc.tile_pool(name="pdn", bufs=1, space="PSUM") as pdn,
            tc.tile_pool(name="ptr", bufs=2, space="PSUM") as ptr,
        ):
            # ---- load inputs ----
            xsb = big.tile([P, KT * T], BF16)
            wup = big.tile([P, KT * E * 256], BF16)
            wdn = big.tile([P, E * H], BF16)
            lg = big.tile([P, NT * E], F32)
            nc.sync.dma_start(wup[:], wup_ext[:])
            nc.sync.dma_start(wdn[:], wdn_ext[:])
            nc.sync.dma_start(lg[:], lg_ext[:])
            # split token load by tile groups so tile 0 starts early
            XG = 4
            for c in range(XG):
                w = T // XG
                nc.sync.dma_start(
                    xsb[:].rearrange("p (k t) -> p k t", k=KT)[:, :, c * w:(c + 1) * w],
                    xt_ext[:].rearrange("p (k t) -> p k t", k=KT)[:, :, c * w:(c + 1) * w])

            ident = big.tile([P, P], BF16)
            make_identity(nc, ident[:])

            # ---- routing: w8[t, e] combine weights ----
            lg3 = lg[:].rearrange("p (j e) -> p j e", e=E)
            m1 = big.tile([P, NT], F32)
            m2 = big.tile([P, NT], F32)
            eq1 = big.tile([P, NT * E], F32)
            tmp = big.tile([P, NT * E], F32)
            w8 = big.tile([P, NT * E], F32)

            nc.vector.reduce_max(m1[:].unsqueeze(-1), lg3, axis=AX.X)
            m1b = m1[:].unsqueeze(-1).to_broadcast([P, NT, E])
            eq13 = eq1[:].rearrange("p (j e) -> p j e", e=E)
            nc.vector.tensor_tensor(eq13, lg3, m1b, op=OP.is_equal)
            # tmp = lg - 1e30*eq1  (mask out the argmax)
            tmp3 = tmp[:].rearrange("p (j e) -> p j e", e=E)
            nc.vector.tensor_scalar(tmp3, eq13, -1e30, None, op0=OP.mult)
            nc.vector.tensor_tensor(tmp3, tmp3, lg3, op=OP.add)
            nc.vector.reduce_max(m2[:].unsqueeze(-1), tmp3, axis=AX.X)
            m2b = m2[:].unsqueeze(-1).to_broadcast([P, NT, E])

            # denom_recip = 1 / (1 + exp(m2 - m1))
            dr = big.tile([P, NT], F32)
            nc.vector.tensor_tensor(dr[:], m2[:], m1[:], op=OP.subtract)
            nc.scalar.activation(dr[:], dr[:], AF.Exp)
            nc.vector.tensor_scalar(dr[:], dr[:], 1.0, None, op0=OP.add)
            nc.vector.reciprocal(dr[:], dr[:])

            # w8 = exp(lg - m1) * (lg >= m2) * denom_recip
            w83 = w8[:].rearrange("p (j e) -> p j e", e=E)
            nc.vector.tensor_tensor(w83, lg3, m1b, op=OP.subtract)
            nc.scalar.activation(w8[:], w8[:], AF.Exp)
            nc.vector.tensor_tensor(tmp3, lg3, m2b, op=OP.is_ge)
            nc.vector.tensor_tensor(w83, w83, tmp3, op=OP.mult)
            drb = dr[:].unsqueeze(-1).to_broadcast([P, NT, E])
            nc.vector.tensor_tensor(w83, w83, drb, op=OP.mult)

            # ---- main loop over token tiles ----
            # experts processed in two groups of 4 with separate PSUM banks so
            # group g+1's up-GEMM overlaps group g's activation/down phase
            for j in range(NT):
                po = [pdn.tile([P, 512], F32, tag="po%d" % q, name="po%d_%d" % (q, j)) for q in range(2)]
                for g in range(2):
                    pu = [pup.tile([P, 512], F32, tag="pu%d_%d" % (g, q),
                                   name="pu%d_%d_%d" % (g, q, j)) for q in range(2)]
                    for k in range(KT):
                        lhsT = xsb[:, k * T + j * P: k * T + (j + 1) * P]
                        for q in range(2):
                            eq = 4 * g + 2 * q
                            nc.tensor.matmul(
                                pu[q][:],
                                lhsT,
                                wup[:, (k * E + eq) * 256:(k * E + eq + 2) * 256],
                                start=(k == 0),
                                stop=(k == KT - 1),
                            )
                    for ei in range(4):
                        e = 4 * g + ei
                        gu = pu[ei // 2][:, (ei % 2) * 256:(ei % 2) * 256 + 256]
                        sig = work.tile([P, IS], F32, tag="sig")
                        nc.scalar.activation(sig[:], gu[:, 0:IS], AF.Sigmoid)
                        nc.vector.tensor_tensor(sig[:], sig[:], gu[:, 0:IS],
                                                op=OP.mult)
                        hg = work.tile([P, IS], BF16, tag="hg")
                        # hg = (sig*gate) * w8 * up  in one fused pass
                        nc.vector.scalar_tensor_tensor(
                            hg[:], sig[:], w8[:, j * E + e: j * E + e + 1],
                            gu[:, IS:2 * IS], op0=OP.mult, op1=OP.mult)
                        ptr_t = ptr.tile([P, P], BF16, tag="ptr")
                        nc.tensor.transpose(ptr_t[:], hg[:], ident[:])
                        hgT = work.tile([P, P], BF16, tag="hgT")
                        nc.vector.tensor_copy(hgT[:], ptr_t[:])
                        for half in range(2):
                            nc.tensor.matmul(
                                po[half][:],
                                hgT[:],
                                wdn[:, e * H + half * 512: e * H + (half + 1) * 512],
                                start=(e == 0),
                                stop=(e == E - 1),
                            )
                ot = outp.tile([P, H], BF16, tag="ot")
                nc.scalar.copy(ot[:, 0:512], po[0][:])
                nc.vector.tensor_copy(ot[:, 512:1024], po[1][:])
                nc.sync.dma_start(rs_in[j * P:(j + 1) * P, :], ot[:])

            # ---- chunked reduce-scatter (overlaps tail of compute) ----
            RSC = 4 if T % (4 * 8 * P) == 0 else 1
            CHR = T // RSC
            for c in range(RSC):
                nc.gpsimd.collective_compute(
                    "ReduceScatter",
                    OP.add,
                    replica_groups=[list(range(N_CORES))],
                    ins=[rs_in[c * CHR:(c + 1) * CHR, :].opt()],
                    outs=[rs_out[c * (CHR // N_CORES):
                                 (c + 1) * (CHR // N_CORES), :].opt()],
                )

            # ---- convert own shard to f32 and write out ----
            PR = min(TS, P)
            CT = TS // PR
            ob = outp.tile([PR, CT * H], BF16, tag="ob")
            of = outp.tile([PR, CT * H], F32, tag="of")
            nc.sync.dma_start(
                ob[:].rearrange("p (c f) -> p c f", f=H),
                rs_out[:].rearrange("(c p) f -> p c f", p=PR),
            )
            nc.vector.tensor_copy(of[:], ob[:])
            nc.sync.dma_start(
                out_ext[:].rearrange("(c p) f -> p c f", p=PR),
                of[:].rearrange("p (c f) -> p c f", f=H),
            )

    nc.compile()
    return nc


def make_in_maps(hidden_states, router_logits, up_weight, down_weight):
    """Host-side sharding/layout prep. Returns per-core input dicts."""
    T = hidden_states.shape[0]
    bf = ml_dtypes.bfloat16
    x16 = hidden_states.astype(bf)
    xt = _rearrange(x16, "t (k p) -> p (k t)", p=P)
    lg = _rearrange(router_logits.astype(np.float32), "(j p) e -> p (j e)", p=P)
    in_maps = []
    for m in range(N_CORES):
        gate = up_weight[:, :, m * IS:(m + 1) * IS]
        up = up_weight[:, :, I_FULL + m * IS: I_FULL + (m + 1) * IS]
        wcat = np.concatenate([gate, up], axis=2).astype(bf)  # [E, H, 256]
        wup = _rearrange(wcat, "e (k p) c -> p (k e c)", p=P)
        wdn = _rearrange(
            down_weight[:, m * IS:(m + 1) * IS, :].astype(bf), "e i f -> i (e f)")
        in_maps.append({"xt": xt, "wup": wup, "wdn": wdn, "lg": lg})
    return in_maps


_GRAPH_CACHE = {}


def _get_graph(T):
    if T not in _GRAPH_CACHE:
        _GRAPH_CACHE[T] = build_graph(T)
    return _GRAPH_CACHE[T]


def kernel(hidden_states, router_logits, up_weight, down_weight, topk,
           trace=False):
    assert int(topk) == K_TOP
    hidden_states = np.asarray(hidden_states, dtype=np.float32)
    router_logits = np.asarray(router_logits, dtype=np.float32)
    up_weight = np.asarray(up_weight, dtype=np.float32)
    down_weight = np.asarray(down_weight, dtype=np.float32)
    T = hidden_states.shape[0]
    nc = _get_graph(T)
    in_maps = make_in_maps(hidden_states, router_logits, up_weight, down_weight)
    res = run_bass_kernel_spmd(nc, in_maps, list(range(N_CORES)), trace=trace)
    TS = T // N_CORES
    RSC = 4 if T % (4 * 8 * P) == 0 else 1
    CHR = T // RSC
    SS = CHR // N_CORES
    out = np.empty((T, H), dtype=np.float32)
    for r in range(N_CORES):
        o = res.results[r]["out"]
        for c in range(RSC):
            out[c * CHR + r * SS: c * CHR + (r + 1) * SS] = o[c * SS:(c + 1) * SS]
    kernel.last_exec_time_ns = res.exec_time_ns
    return out


kernel.last_exec_time_ns = None


# revision 17
# speedup vs baseline: 1.0190x; 1.0190x over previous
"""Trainium2 Bass kernel for top-2-of-8 MoE (T=4096, H=1024, I=1024).

Strategy (tensor-parallel over intermediate dim, 8 cores):
  - Each core gets the full tokens (replicated) + a 128-wide shard of every
    expert's up/down projection (I is sharded 8 ways).
  - Routing (softmax + top-2 + renormalize) is computed on every core from the
    full router logits; it reduces to per-(token, expert) combine weights
    w8[t,e] (nonzero only for the 2 selected experts).
  - Dense-masked grouped GEMM: for each 128-token tile, compute all 8 experts'
    up-proj (tokens stationary), gated-SiLU, scale by w8, transpose, down-proj
    accumulated over experts into PSUM -> per-core partial output [T, H].
  - ReduceScatter(add) over the 8 cores sums the I-shard partials; core r ends
    with rows [r*T/8, (r+1)*T/8) which the host concatenates.

Compute dtype bf16 (f32 PSUM accumulation), f32 output.
"""

import os
import sys

for _p in ("/opt/trn_rl_repo",):
    if _p not in sys.path:
        sys.path.append(_p)

import numpy as np
import ml_dtypes

import concourse.bass as bass
import concourse.bacc as bacc
import concourse.mybir as mybir
import concourse.tile as tile
from concourse.bass_utils import run_bass_kernel_spmd
from concourse.masks import make_identity

BF16 = mybir.dt.bfloat16
F32 = mybir.dt.float32
AX = mybir.AxisListType
OP = mybir.AluOpType
AF = mybir.ActivationFunctionType

N_CORES = 8
H = 1024
I_FULL = 1024
E = 8
K_TOP = 2
IS = I_FULL // N_CORES  # 128, per-core shard of intermediate dim
KT = H // 128  # 8 contraction k-tiles
P = 128


def _rearrange(x, pattern, **kw):
    import einops

    return np.ascontiguousarray(einops.rearrange(x, pattern, **kw))


def build_graph(T):
    """Build the SPMD graph for a T-token problem. Same graph on all cores."""
    NT = T // P  # token tiles
    TS = T // N_CORES  # output rows per core

    nc = bacc.Bacc("TRN2", target_bir_lowering=False, debug=False,
                   num_devices=N_CORES)

    xt_ext = nc.dram_tensor("xt", [P, KT * T], BF16, kind="ExternalInput")
    wup_ext = nc.dram_tensor("wup", [P, KT * E * 256], BF16, kind="ExternalInput")
    wdn_ext = nc.dram_tensor("wdn", [P, E * H], BF16, kind="ExternalInput")
    lg_ext = nc.dram_tensor("lg", [P, NT * E], F32, kind="ExternalInput")
    out_ext = nc.dram_tensor("out", [TS, H], F32, kind="ExternalOutput")

    # internal DRAM for the collective
    rs_in = nc.dram_tensor("rs_in", [T, H], BF16)
    rs_out = nc.dram_tensor("rs_out", [TS, H], BF16)

    with tile.TileContext(nc) as tc:
        with (
            tc.tile_pool(name="big", bufs=1) as big,
            tc.tile_pool(name="work", bufs=3) as work,
            tc.tile_pool(name="outp", bufs=2) as outp,
            tc.tile_pool(name="pup", bufs=1, space="PSUM") as pup,  # 4 tags x 1
            tc.tile_pool(name="pdn", bufs=1, space="PSUM") as pdn,
            tc.tile_pool(name="ptr", bufs=2, space="PSUM") as ptr,
        ):
            # ---- load inputs ----
            xsb = big.tile([P, KT * T], BF16)
            wup = big.tile([P, KT * E * 256], BF16)
            wdn = big.tile([P, E * H], BF16)
            lg = big.tile([P, NT * E], F32)
            WKC = KT * E * 256 // 4
            for c in range(4):
                nc.sync.dma_start(wup[:, c * WKC:(c + 1) * WKC],
                                  wup_ext[:, c * WKC:(c + 1) * WKC])
            nc.sync.dma_start(wdn[:], wdn_ext[:])
            nc.sync.dma_start(lg[:], lg_ext[:])
            # split token load by tile groups so tile 0 starts early
            XG = 4
            for c in range(XG):
                w = T // XG
                nc.sync.dma_start(
                    xsb[:].rearrange("p (k t) -> p k t", k=KT)[:, :, c * w:(c + 1) * w],
                    xt_ext[:].rearrange("p (k t) -> p k t", k=KT)[:, :, c * w:(c + 1) * w])

            ident = big.tile([P, P], BF16)
            make_identity(nc, ident[:])

            # ---- routing: w8[t, e] combine weights ----
            lg3 = lg[:].rearrange("p (j e) -> p j e", e=E)
            m1 = big.tile([P, NT], F32)
            m2 = big.tile([P, NT], F32)
            eq1 = big.tile([P, NT * E], F32)
            tmp = big.tile([P, NT * E], F32)
            w8 = big.tile([P, NT * E], F32)

            nc.vector.reduce_max(m1[:].unsqueeze(-1), lg3, axis=AX.X)
            m1b = m1[:].unsqueeze(-1).to_broadcast([P, NT, E])
            eq13 = eq1[:].rearrange("p (j e) -> p j e", e=E)
            nc.vector.tensor_tensor(eq13, lg3, m1b, op=OP.is_equal)
            # tmp = lg - 1e30*eq1  (mask out the argmax)
            tmp3 = tmp[:].rearrange("p (j e) -> p j e", e=E)
            nc.vector.tensor_scalar(tmp3, eq13, -1e30, None, op0=OP.mult)
            nc.vector.tensor_tensor(tmp3, tmp3, lg3, op=OP.add)
            nc.vector.reduce_max(m2[:].unsqueeze(-1), tmp3, axis=AX.X)
            m2b = m2[:].unsqueeze(-1).to_broadcast([P, NT, E])

            # denom_recip = 1 / (1 + exp(m2 - m1))
            dr = big.tile([P, NT], F32)
            nc.vector.tensor_tensor(dr[:], m2[:], m1[:], op=OP.subtract)
            nc.scalar.activation(dr[:], dr[:], AF.Exp)
            nc.vector.tensor_scalar(dr[:], dr[:], 1.0, None, op0=OP.add)
            nc.vector.reciprocal(dr[:], dr[:])

            # w8 = exp(lg - m1) * (lg >= m2) * denom_recip
            w83 = w8[:].rearrange("p (j e) -> p j e", e=E)
            nc.vector.tensor_tensor(w83, lg3, m1b, op=OP.subtract)
            nc.scalar.activation(w8[:], w8[:], AF.Exp)
            nc.vector.tensor_tensor(tmp3, lg3, m2b, op=OP.is_ge)
            nc.vector.tensor_tensor(w83, w83, tmp3, op=OP.mult)
            drb = dr[:].unsqueeze(-1).to_broadcast([P, NT, E])
            nc.vector.tensor_tensor(w83, w83, drb, op=OP.mult)

            # ---- main loop over token tiles ----
            # experts processed in two groups of 4 with separate PSUM banks so
            # group g+1's up-GEMM overlaps group g's activation/down phase
            for j in range(NT):
                po = [pdn.tile([P, 512], F32, tag="po%d" % q, name="po%d_%d" % (q, j)) for q in range(2)]
                for g in range(2):
                    pu = [pup.tile([P, 512], F32, tag="pu%d_%d" % (g, q),
                                   name="pu%d_%d_%d" % (g, q, j)) for q in range(2)]
                    for k in range(KT):
                        lhsT = xsb[:, k * T + j * P: k * T + (j + 1) * P]
                        for q in range(2):
                            eq = 4 * g + 2 * q
                            nc.tensor.matmul(
                                pu[q][:],
                                lhsT,
                                wup[:, (k * E + eq) * 256:(k * E + eq + 2) * 256],
                                start=(k == 0),
                                stop=(k == KT - 1),
                            )
                    for ei in range(4):
                        e = 4 * g + ei
                        gu = pu[ei // 2][:, (ei % 2) * 256:(ei % 2) * 256 + 256]
                        sig = work.tile([P, IS], F32, tag="sig")
                        nc.scalar.activation(sig[:], gu[:, 0:IS], AF.Sigmoid)
                        nc.vector.tensor_tensor(sig[:], sig[:], gu[:, 0:IS],
                                                op=OP.mult)
                        hg = work.tile([P, IS], BF16, tag="hg")
                        # hg = (sig*gate) * w8 * up  in one fused pass
                        nc.vector.scalar_tensor_tensor(
                            hg[:], sig[:], w8[:, j * E + e: j * E + e + 1],
                            gu[:, IS:2 * IS], op0=OP.mult, op1=OP.mult)
                        ptr_t = ptr.tile([P, P], BF16, tag="ptr")
                        nc.tensor.transpose(ptr_t[:], hg[:], ident[:])
                        hgT = work.tile([P, P], BF16, tag="hgT")
                        nc.vector.tensor_copy(hgT[:], ptr_t[:])
                        for half in range(2):
                            nc.tensor.matmul(
                                po[half][:],
                                hgT[:],
                                wdn[:, e * H + half * 512: e * H + (half + 1) * 512],
                                start=(e == 0),
                                stop=(e == E - 1),
                            )
                ot = outp.tile([P, H], BF16, tag="ot")
                nc.scalar.copy(ot[:, 0:512], po[0][:])
                nc.vector.tensor_copy(ot[:, 512:1024], po[1][:])
                nc.sync.dma_start(rs_in[j * P:(j + 1) * P, :], ot[:])

            # ---- chunked reduce-scatter (overlaps tail of compute) ----
            RSC = 4 if T % (4 * 8 * P) == 0 else 1
            CHR = T // RSC
            for c in range(RSC):
                nc.gpsimd.collective_compute(
                    "ReduceScatter",
                    OP.add,
                    replica_groups=[list(range(N_CORES))],
                    ins=[rs_in[c * CHR:(c + 1) * CHR, :].opt()],
                    outs=[rs_out[c * (CHR // N_CORES):
                                 (c + 1) * (CHR // N_CORES), :].opt()],
                )

            # ---- convert own shard to f32 per RS chunk (pipelined) ----
            PR = min(TS, P)
            CT = TS // PR
            for c in range(CT):
                ob = outp.tile([PR, H], BF16, tag="ob", name="ob%d" % c)
                of = outp.tile([PR, H], F32, tag="of", name="of%d" % c)
                nc.sync.dma_start(ob[:], rs_out[c * PR:(c + 1) * PR, :])
                nc.vector.tensor_copy(of[:], ob[:])
                nc.sync.dma_start(out_ext[c * PR:(c + 1) * PR, :], of[:])

    nc.compile()
    return nc


def make_in_maps(hidden_states, router_logits, up_weight, down_weight):
    """Host-side sharding/layout prep. Returns per-core input dicts."""
    T = hidden_states.shape[0]
    bf = ml_dtypes.bfloat16
    x16 = hidden_states.astype(bf)
    xt = _rearrange(x16, "t (k p) -> p (k t)", p=P)
    lg = _rearrange(router_logits.astype(np.float32), "(j p) e -> p (j e)", p=P)
    in_maps = []
    for m in range(N_CORES):
        gate = up_weight[:, :, m * IS:(m + 1) * IS]
        up = up_weight[:, :, I_FULL + m * IS: I_FULL + (m + 1) * IS]
        wcat = np.concatenate([gate, up], axis=2).astype(bf)  # [E, H, 256]
        wup = _rearrange(wcat, "e (k p) c -> p (k e c)", p=P)
        wdn = _rearrange(
            down_weight[:, m * IS:(m + 1) * IS, :].astype(bf), "e i f -> i (e f)")
        in_maps.append({"xt": xt, "wup": wup, "wdn": wdn, "lg": lg})
    return in_maps


_GRAPH_CACHE = {}


def _get_graph(T):
    if T not in _GRAPH_CACHE:
        _GRAPH_CACHE[T] = build_graph(T)
    return _GRAPH_CACHE[T]


def kernel(hidden_states, router_logits, up_weight, down_weight, topk,
           trace=False):
    assert int(topk) == K_TOP
    hidden_states = np.asarray(hidden_states, dtype=np.float32)
    router_logits = np.asarray(router_logits, dtype=np.float32)
    up_weight = np.asarray(up_weight, dtype=np.float32)
    down_weight = np.asarray(down_weight, dtype=np.float32)
    T = hidden_states.shape[0]
    nc = _get_graph(T)
    in_maps = make_in_maps(hidden_states, router_logits, up_weight, down_weight)
    res = run_bass_kernel_spmd(nc, in_maps, list(range(N_CORES)), trace=trace)
    TS = T // N_CORES
    RSC = 4 if T % (4 * 8 * P) == 0 else 1
    CHR = T // RSC
    SS = CHR // N_CORES
    out = np.empty((T, H), dtype=np.float32)
    for r in range(N_CORES):
        o = res.results[r]["out"]
        for c in range(RSC):
            out[c * CHR + r * SS: c * CHR + (r + 1) * SS] = o[c * SS:(c + 1) * SS]
    kernel.last_exec_time_ns = res.exec_time_ns
    return out


kernel.last_exec_time_ns = None
